# revision 13
# baseline (speedup 1.0000x reference)
"""Trainium2 Bass kernel for the non-local (self-attention over spatial
positions) block.

Per batch b (8 batches -> one per NeuronCore):
    xf    = x[b]                       [C=128, N=4096]
    theta = w_theta @ xf               [64, N]
    phi   = w_phi   @ xf               [64, N]
    g     = w_g     @ xf               [64, N]
    attn  = softmax(theta^T phi)       [N, N]   (softmax over keys m)
    y     = g @ attn^T                 [64, N]
    out   = w_last @ y + xf            [128, N]

Design (per core), v2:
 - All matmul moving operands are f32r (1 cycle/col) or bf16; no f32
   LOW_HIGH dual-pass anywhere.
 - scoresT[m, q] orientation (phi tiles stationary) so exp(scoresT)
   feeds the y matmul directly as the moving operand.
 - One [128, 512] PSUM score tile per 128-m-tile; 5-deep score pool so
   the PE runs ~3 tiles ahead of the y-matmul consumer and never
   stalls -> stays at max DVFS p-state (2.4 GHz) instead of 1.2.
 - exp is the #2 engine load (16.8M elements): split 20/12 between the
   ACT engine (real Exp) and the DVE (Schraudolph bit-trick exp: one
   tensor_scalar affine with int32 output, bitcast back to f32r).
   Rel-err of the trick ~2-3% on 12/32 of key tiles -> ~7e-3 end to
   end, inside the 2e-2 budget.
 - Row sums come free from a ones column appended to gT (stationary of
   the y matmul); normalization uses reciprocal_approx_fast + gpsimd
   partition_broadcast, multiply straight out of PSUM.
 - Input DMA / projections / first q-chunk are fused so the PE warms up
   while the input streams in; weights are DMA'd before the bulk input.
"""

import sys

import numpy as np

for _p in ("/opt/trn_rl_repo",):
    if _p not in sys.path:
        sys.path.insert(0, _p)

import concourse.bass as bass
from concourse import bacc
import concourse.mybir as mybir
import concourse.tile as tile
from concourse.alu_op_type import AluOpType
from concourse.bass_utils import run_bass_kernel_spmd

F32 = mybir.dt.float32
F32R = mybir.dt.float32r
BF16 = mybir.dt.bfloat16
I32 = mybir.dt.int32

P = 128     # channels C / partition dim
CB = 64     # bottleneck channels
NQ = 4096   # spatial positions (64*64)
NMT = 32    # m (key) tiles of 128
NQC = 8     # q chunks of 512

# Schraudolph exp in bf16: exp(s) ~= bitcast<bf16>(int16(A*s + B)).
# bf16 keeps the y matmul free of the verifier's fp32r rounding rule.
EXP_A = float(2**7 / np.log(2.0))
EXP_B = float(127.0 * 2**7 - 6.0)
# m-tiles with mi % 8 < ACT_SHARE go to the ACT engine, rest to DVE
ACT_SHARE = 5

_NC_CACHE = {}


def _build():
    nc = bacc.Bacc()
    x_in = nc.declare_dram_parameter("xb", [P, NQ], F32, isOutput=False)
    wqk_in = nc.declare_dram_parameter("wqk", [P, P], F32, isOutput=False)
    wg_in = nc.declare_dram_parameter("wgT", [P, CB], F32, isOutput=False)
    wl_in = nc.declare_dram_parameter("wl", [CB, P], F32, isOutput=False)
    out_d = nc.declare_dram_parameter("out", [P, NQ], F32, isOutput=True)

    with tile.TileContext(nc) as tc:
        with (
            tc.tile_pool(name="const", bufs=1) as const,
            tc.tile_pool(name="big", bufs=1) as big,
            tc.tile_pool(name="work", bufs=2) as work,
            tc.tile_pool(name="probs", bufs=6) as probs,
            tc.tile_pool(name="spool", bufs=5, space="PSUM") as spool,
            tc.tile_pool(name="ypool", bufs=2, space="PSUM") as ypool,
            tc.tile_pool(name="epool", bufs=1, space="PSUM") as epool,
        ):
            # ---- small weights first (needed before any compute) ----
            # f32r matmul operands must be produced by an engine that rounds
            # to f32r (the DVE) -- DMA-raw f32 bits fed to an f32r matmul
            # compute garbage on hardware even though CoreSim accepts them.
            wqk_f = const.tile([P, P], F32)
            wg_f = const.tile([P, CB], F32)
            wl_f = const.tile([CB, P], F32)
            nc.sync.dma_start(out=wqk_f, in_=wqk_in[:, :])
            nc.sync.dma_start(out=wg_f, in_=wg_in[:, :])
            nc.sync.dma_start(out=wl_f, in_=wl_in[:, :])
            wqk = const.tile([P, P], F32R)
            wg = const.tile([P, CB], F32R)
            wl = const.tile([CB, P], F32R)
            nc.vector.tensor_copy(wqk, wqk_f)
            nc.vector.tensor_copy(wg, wg_f)
            nc.vector.tensor_copy(wl, wl_f)

            xb = big.tile([P, NQ], F32)
            xbr = big.tile([P, NQ], F32R)
            # theta/phi both at base partition 0 (matmul requires stationary
            # and moving operands to share a base partition)
            theta = big.tile([CB, NQ], F32R)
            phi = big.tile([CB, NQ], F32R)
            # gT in 65-col slots; col 64 = ones for the row-sum trick
            gt = big.tile([P, NMT * (CB + 1)], BF16)
            nc.vector.memset(gt, 1.0)
            gt3 = gt.rearrange("p (m c) -> p m c", c=CB + 1)

            # ---------------- pipelined helpers ----------------
            qof = [qc * 512 for qc in range(NQC)]

            def score_mm(qc, mi):
                sp = spool.tile([P, 512], F32, tag="s")
                nc.tensor.matmul(
                    sp,
                    phi[:, mi * 128:(mi + 1) * 128],
                    theta[:, qof[qc]:qof[qc] + 512],
                    start=True, stop=True,
                )
                return sp

            def exp_tile(mi, sp):
                pb = probs.tile([P, 512], BF16, tag="pb")
                if mi % 8 < ACT_SHARE:
                    nc.scalar.activation(
                        pb, sp, mybir.ActivationFunctionType.Exp
                    )
                else:
                    nc.vector.tensor_scalar(
                        pb.bitcast(mybir.dt.int16), sp, EXP_A, EXP_B,
                        AluOpType.mult, AluOpType.add,
                    )
                return pb

            def y_mm(yps, mi, pb):
                nc.tensor.matmul(
                    yps,
                    gt[:, mi * (CB + 1):(mi + 1) * (CB + 1)],
                    pb,
                    start=(mi == 0), stop=(mi == NMT - 1),
                )

            # per-chunk epilogue, split in two so the op matmul can be
            # deferred into the next chunk's PE stream
            def epi_head(qc, yps):
                yu = work.tile([CB + 1, 512], F32R, tag="yu")
                nc.vector.tensor_copy(yu, yps)          # frees yps slot
                rinv = work.tile([1, 512], F32, tag="rinv")
                nc.vector.reciprocal(rinv, yu.bitcast(F32)[CB:CB + 1, :])
                rb = work.tile([P, 512], F32, tag="rb")
                nc.gpsimd.partition_broadcast(rb, rinv)
                return yu, rb

            def epi_mm(state):
                qc, yu, rb = state
                op = epool.tile([P, 512], F32, tag="op")
                nc.tensor.matmul(op, wl, yu[0:CB, :], start=True, stop=True)
                return (qc, op, rb)

            def epi_tail(state):
                qc, op, rb = state
                ob = work.tile([P, 512], F32, tag="ob")
                nc.vector.tensor_mul(ob, op, rb)
                ob2 = work.tile([P, 512], F32, tag="ob2")
                nc.vector.tensor_add(
                    ob2, ob, xb[:, qof[qc]:qof[qc] + 512]
                )
                nc.sync.dma_start(
                    out=out_d[:, qof[qc]:qof[qc] + 512], in_=ob2
                )

            # ---------------- init fused with q-chunk 0 ----------------
            # Per 512-col xb chunk j: DMA, theta/phi projection, 4 gT
            # projections; from j>=1 also run q-chunk-0 score/exp/y for
            # the m-tiles whose phi/gt landed in iteration j-1.
            LOOK = 3  # y matmul runs LOOK score-tiles behind
            yps0 = ypool.tile([CB + 1, 512], F32, tag="y")
            pbq = {}  # mi -> pb tile awaiting its y matmul

            def chunk0_tiles(mlo, mhi):
                for mi in range(mlo, mhi):
                    sp = score_mm(0, mi)
                    pbq[mi] = exp_tile(mi, sp)
                    if mi - LOOK >= 0:
                        y_mm(yps0, mi - LOOK, pbq.pop(mi - LOOK))

            for j in range(8):
                cs = slice(j * 512, (j + 1) * 512)
                nc.sync.dma_start(out=xb[:, cs], in_=x_in[:, cs])
                nc.vector.tensor_copy(xbr[:, cs], xb[:, cs])
                ps = spool.tile([P, 512], F32, tag="s")
                nc.tensor.matmul(ps, wqk, xbr[:, cs], start=True, stop=True)
                nc.vector.tensor_copy(theta[:, cs], ps[0:CB, :])
                nc.vector.tensor_copy(phi[:, cs], ps[CB:P, :])
                gp = spool.tile([P, 512], F32, tag="s")
                gp3 = gp.rearrange("p (m c) -> p m c", c=CB)
                for k in range(4):
                    mi = 4 * j + k
                    nc.tensor.matmul(
                        gp3[:, k, :], xbr[:, mi * 128:(mi + 1) * 128], wg,
                        start=True, stop=True,
                    )
                nc.scalar.copy(
                    gt3[:, 4 * j:4 * j + 4, 0:CB], gp3[:, 0:4, :]
                )
                if j >= 1:
                    chunk0_tiles(4 * (j - 1), 4 * j)
            chunk0_tiles(28, 32)
            for mi in range(NMT - LOOK, NMT):
                y_mm(yps0, mi, pbq.pop(mi))
            pending = (0,) + epi_head(0, yps0)

            # ---------------- q-chunks 1..7 ----------------
            for qc in range(1, NQC):
                yps = ypool.tile([CB + 1, 512], F32, tag="y")
                for mi in range(NMT):
                    sp = score_mm(qc, mi)
                    pbq[mi] = exp_tile(mi, sp)
                    if mi == 4 and pending is not None:
                        pending = epi_mm(pending)
                    if mi == 6 and pending is not None:
                        epi_tail(pending)
                        pending = None
                    if mi - LOOK >= 0:
                        y_mm(yps, mi - LOOK, pbq.pop(mi - LOOK))
                for mi in range(NMT - LOOK, NMT):
                    y_mm(yps, mi, pbq.pop(mi))
                pending = (qc,) + epi_head(qc, yps)

            epi_tail(epi_mm(pending))

    nc.finalize()
    return nc


def kernel(x, w_theta, w_phi, w_g, w_last):
    B, C, H, W = x.shape
    N = H * W
    xf = np.ascontiguousarray(x.reshape(B, C, N), dtype=np.float32)
    wqk = np.ascontiguousarray(
        np.concatenate([w_theta.T, w_phi.T], axis=1), dtype=np.float32
    )
    wgT = np.ascontiguousarray(w_g.T, dtype=np.float32)
    wl = np.ascontiguousarray(w_last.T, dtype=np.float32)

    if "nc" not in _NC_CACHE:
        _NC_CACHE["nc"] = _build()
    nc = _NC_CACHE["nc"]

    in_maps = [
        {"xb": xf[b], "wqk": wqk, "wgT": wgT, "wl": wl} for b in range(B)
    ]
    r = run_bass_kernel_spmd(nc, in_maps, list(range(B)))
    out = np.stack([r.results[b]["out"] for b in range(B)], axis=0)
    return out.reshape(B, C, H, W).astype(np.float32)


# revision 14
# speedup vs baseline: 1.2296x; 1.2296x over previous
"""Trainium2 Bass kernel for the non-local (self-attention over spatial
positions) block.

Per batch b (8 batches -> one per NeuronCore):
    xf    = x[b]                       [C=128, N=4096]
    theta = w_theta @ xf               [64, N]
    phi   = w_phi   @ xf               [64, N]
    g     = w_g     @ xf               [64, N]
    attn  = softmax(theta^T phi)       [N, N]   (softmax over keys m)
    y     = g @ attn^T                 [64, N]
    out   = w_last @ y + xf            [128, N]

Design (per core), v5:
 - scoresT[m, q] orientation (phi tiles stationary) so exp(scoresT)
   feeds the y matmul directly as the moving operand.
 - Scores contract only 64 channels, so theta/phi are duplicated into
   both partition halves and each pair of m-tiles runs as two
   concurrent matmuls in disjoint PE row groups (~1.6x at the
   sustained 1.2 GHz p-state vs sequential 64-row matmuls).
 - f32r moving operands (1 cycle/col); every f32r matmul operand is
   produced by a DVE/ACT copy — DMA-raw f32 bits fed to an f32r
   matmul compute garbage on hardware even though CoreSim passes.
 - exp split 20/12 between ACT (real Exp) and DVE (Schraudolph
   bit-trick: one tensor_scalar affine with int16 output bitcast as
   bf16). The y matmul is uniform bf16 (gt stationary, probs moving).
 - Row sums via a ones column in gt; normalization: DVE reciprocal +
   gpsimd partition_broadcast; multiply straight out of PSUM; the
   w_last matmul of chunk c is deferred into chunk c+1's PE stream.
 - Input DMA / projections / first q-chunk fused so the PE starts
   while the input streams in; weights DMA'd before the bulk input.
"""

import sys

import numpy as np

for _p in ("/opt/trn_rl_repo",):
    if _p not in sys.path:
        sys.path.insert(0, _p)

import concourse.bass as bass
from concourse import bacc
import concourse.mybir as mybir
import concourse.tile as tile
from concourse.alu_op_type import AluOpType
from concourse.bass_utils import run_bass_kernel_spmd

F32 = mybir.dt.float32
F32R = mybir.dt.float32r
BF16 = mybir.dt.bfloat16
I16 = mybir.dt.int16

P = 128     # channels C / partition dim
CB = 64     # bottleneck channels
NQ = 4096   # spatial positions (64*64)
NMT = 32    # m (key) tiles of 128
NPAIR = 16  # m-tile pairs per q chunk
NQC = 8     # q chunks of 512

# Schraudolph exp in bf16: exp(s) ~= bitcast<bf16>(int16(A*s + B))
EXP_A = float(2**7 / np.log(2.0))
EXP_B = float(127.0 * 2**7 - 6.0)
# m-tiles with mi % 8 < ACT_SHARE go to the ACT engine, rest to DVE
ACT_SHARE = 5

_NC_CACHE = {}


def _build():
    nc = bacc.Bacc()
    x_in = nc.declare_dram_parameter("xb", [P, NQ], F32, isOutput=False)
    wqk_in = nc.declare_dram_parameter("wqk", [P, P], F32, isOutput=False)
    wg_in = nc.declare_dram_parameter("wgT", [P, CB], F32, isOutput=False)
    wl_in = nc.declare_dram_parameter("wl", [CB, P], F32, isOutput=False)
    out_d = nc.declare_dram_parameter("out", [P, NQ], F32, isOutput=True)

    with tile.TileContext(nc) as tc:
        with (
            tc.tile_pool(name="const", bufs=1) as const,
            tc.tile_pool(name="big", bufs=1) as big,
            tc.tile_pool(name="work", bufs=2) as work,
            tc.tile_pool(name="probs", bufs=6) as probs,
            tc.tile_pool(name="spool", bufs=5, space="PSUM") as spool,
            tc.tile_pool(name="ypool", bufs=2, space="PSUM") as ypool,
            tc.tile_pool(name="epool", bufs=1, space="PSUM") as epool,
        ):
            # ---- small weights first (needed before any compute) ----
            wqk_f = const.tile([P, P], F32)
            wg_f = const.tile([P, CB], F32)
            wl_f = const.tile([CB, P], F32)
            nc.sync.dma_start(out=wqk_f, in_=wqk_in[:, :])
            nc.sync.dma_start(out=wg_f, in_=wg_in[:, :])
            nc.sync.dma_start(out=wl_f, in_=wl_in[:, :])
            wqk = const.tile([P, P], F32R)
            wg = const.tile([P, CB], F32R)
            wl = const.tile([CB, P], F32R)
            nc.vector.tensor_copy(wqk, wqk_f)
            nc.vector.tensor_copy(wg, wg_f)
            nc.vector.tensor_copy(wl, wl_f)

            xb = big.tile([P, NQ], F32)
            xbr = big.tile([P, NQ], F32R)
            # theta/phi duplicated into both partition halves so score
            # matmuls for two m-tiles run concurrently in PE row groups
            theta = big.tile([P, NQ], F32R)
            phi = big.tile([P, NQ], F32R)
            # gT in 65-col slots; col 64 = ones for the row-sum trick
            gt = big.tile([P, NMT * (CB + 1)], BF16)
            nc.vector.memset(gt, 1.0)
            gt3 = gt.rearrange("p (m c) -> p m c", c=CB + 1)

            # ---------------- pipelined helpers ----------------
            qof = [qc * 512 for qc in range(NQC)]

            def score_pair(qc, pi):
                """Two concurrent 64-row score matmuls for m-tiles
                2*pi (rows 0:64) and 2*pi+1 (rows 64:128)."""
                q = qof[qc]
                sa = spool.tile([P, 512], F32, tag="s")
                nc.tensor.matmul(
                    sa, phi[0:CB, (2 * pi) * 128:(2 * pi + 1) * 128],
                    theta[0:CB, q:q + 512], start=True, stop=True,
                )
                sb = spool.tile([P, 512], F32, tag="s")
                nc.tensor.matmul(
                    sb, phi[CB:P, (2 * pi + 1) * 128:(2 * pi + 2) * 128],
                    theta[CB:P, q:q + 512], start=True, stop=True,
                )
                return sa, sb

            def exp_tile(mi, sp):
                pb = probs.tile([P, 512], BF16, tag="pb")
                if mi % 8 < ACT_SHARE:
                    nc.scalar.activation(
                        pb, sp, mybir.ActivationFunctionType.Exp
                    )
                else:
                    nc.vector.tensor_scalar(
                        pb.bitcast(I16), sp, EXP_A, EXP_B,
                        AluOpType.mult, AluOpType.add,
                    )
                return pb

            def y_mm(yps, mi, pb):
                nc.tensor.matmul(
                    yps,
                    gt[:, mi * (CB + 1):(mi + 1) * (CB + 1)],
                    pb,
                    start=(mi == 0), stop=(mi == NMT - 1),
                )

            # per-chunk epilogue, split so the w_last matmul can be
            # deferred into the next chunk's PE stream
            def epi_head(qc, yps):
                yu = work.tile([CB + 1, 512], F32R, tag="yu")
                nc.vector.tensor_copy(yu, yps)          # frees yps slot
                rinv = work.tile([1, 512], F32, tag="rinv")
                nc.vector.reciprocal(rinv, yu.bitcast(F32)[CB:CB + 1, :])
                rb = work.tile([P, 512], F32, tag="rb")
                nc.gpsimd.partition_broadcast(rb, rinv)
                return yu, rb

            def epi_mm(state):
                qc, yu, rb = state
                op = epool.tile([P, 512], F32, tag="op")
                nc.tensor.matmul(op, wl, yu[0:CB, :], start=True, stop=True)
                return (qc, op, rb)

            def epi_tail(state):
                qc, op, rb = state
                ob = work.tile([P, 512], F32, tag="ob")
                nc.vector.tensor_mul(ob, op, rb)
                ob2 = work.tile([P, 512], F32, tag="ob2")
                nc.vector.tensor_add(ob2, ob, xb[:, qof[qc]:qof[qc] + 512])
                nc.sync.dma_start(out=out_d[:, qof[qc]:qof[qc] + 512], in_=ob2)

            def pair_tiles(qc, yps, pi, look=2):
                """Scores+exp for pair pi, y matmuls for pair pi-look."""
                sa, sb = score_pair(qc, pi)
                pbq[2 * pi] = exp_tile(2 * pi, sa)
                pbq[2 * pi + 1] = exp_tile(2 * pi + 1, sb)
                pj = pi - look
                if pj >= 0:
                    for mi in (2 * pj, 2 * pj + 1):
                        y_mm(yps, mi, pbq.pop(mi))

            def drain_y(yps, look=2):
                for pj in range(NPAIR - look, NPAIR):
                    for mi in (2 * pj, 2 * pj + 1):
                        y_mm(yps, mi, pbq.pop(mi))

            # ---------------- init fused with q-chunk 0 ----------------
            # Per 512-col xb chunk j: DMA, theta/phi projection, 4 gT
            # projections; from j>=1 also run q-chunk-0 score/exp/y for
            # the m-tile pairs whose phi/gt landed in iteration j-1.
            yps0 = ypool.tile([CB + 1, 512], F32, tag="y")
            pbq = {}  # mi -> pb tile awaiting its y matmul

            for j in range(8):
                cs = slice(j * 512, (j + 1) * 512)
                nc.sync.dma_start(out=xb[:, cs], in_=x_in[:, cs])
                nc.vector.tensor_copy(xbr[:, cs], xb[:, cs])
                ps = spool.tile([P, 512], F32, tag="s")
                nc.tensor.matmul(ps, wqk, xbr[:, cs], start=True, stop=True)
                # theta lower half is partition-aligned -> ACT engine;
                # the shifted copies (DVE only) fill the other halves
                nc.scalar.copy(theta[0:CB, cs], ps[0:CB, :])
                nc.vector.tensor_copy(phi[0:CB, cs], ps[CB:P, :])
                nc.vector.tensor_copy(phi[CB:P, cs], ps[CB:P, :])
                if j == 0:
                    nc.vector.tensor_copy(theta[CB:P, cs], ps[0:CB, :])
                gp = spool.tile([P, 512], F32, tag="s")
                gp3 = gp.rearrange("p (m c) -> p m c", c=CB)
                for k in range(4):
                    mi = 4 * j + k
                    nc.tensor.matmul(
                        gp3[:, k, :], xbr[:, mi * 128:(mi + 1) * 128], wg,
                        start=True, stop=True,
                    )
                nc.scalar.copy(gt3[:, 4 * j:4 * j + 4, 0:CB], gp3[:, 0:4, :])
                if j >= 1:
                    for pi in (2 * (j - 1), 2 * (j - 1) + 1):
                        pair_tiles(0, yps0, pi)
            for pi in (14, 15):
                pair_tiles(0, yps0, pi)
            drain_y(yps0)
            # upper-theta for chunk 1 (needed at its start)
            c1 = slice(512, 1024)
            nc.vector.tensor_copy(theta[CB:P, c1], theta[0:CB, c1])
            pending = (0,) + epi_head(0, yps0)

            # ---------------- q-chunks 1..7 ----------------
            for qc in range(1, NQC):
                yps = ypool.tile([CB + 1, 512], F32, tag="y")
                for pi in range(NPAIR):
                    pair_tiles(qc, yps, pi)
                    if pi == 2 and pending is not None:
                        pending = epi_mm(pending)
                    if pi == 3 and pending is not None:
                        epi_tail(pending)
                        pending = None
                    if pi == 8 and qc < NQC - 1:
                        # upper-theta for the next chunk, off-peak on DVE
                        cn = slice(qof[qc + 1], qof[qc + 1] + 512)
                        nc.vector.tensor_copy(theta[CB:P, cn], theta[0:CB, cn])
                drain_y(yps)
                pending = (qc,) + epi_head(qc, yps)

            epi_tail(epi_mm(pending))

    nc.finalize()
    return nc


def kernel(x, w_theta, w_phi, w_g, w_last):
    B, C, H, W = x.shape
    N = H * W
    xf = np.ascontiguousarray(x.reshape(B, C, N), dtype=np.float32)
    wqk = np.ascontiguousarray(
        np.concatenate([w_theta.T, w_phi.T], axis=1), dtype=np.float32
    )
    wgT = np.ascontiguousarray(w_g.T, dtype=np.float32)
    wl = np.ascontiguousarray(w_last.T, dtype=np.float32)

    if "nc" not in _NC_CACHE:
        _NC_CACHE["nc"] = _build()
    nc = _NC_CACHE["nc"]

    in_maps = [
        {"xb": xf[b], "wqk": wqk, "wgT": wgT, "wl": wl} for b in range(B)
    ]
    r = run_bass_kernel_spmd(nc, in_maps, list(range(B)))
    out = np.stack([r.results[b]["out"] for b in range(B)], axis=0)
    return out.reshape(B, C, H, W).astype(np.float32)


# revision 15
# speedup vs baseline: 1.6945x; 1.3782x over previous
"""Trainium2 Bass kernel for the non-local (self-attention over spatial
positions) block.

Per batch b (8 batches -> one per NeuronCore):
    xf    = x[b]                       [C=128, N=4096]
    theta = w_theta @ xf               [64, N]
    phi   = w_phi   @ xf               [64, N]
    g     = w_g     @ xf               [64, N]
    attn  = softmax(theta^T phi)       [N, N]   (softmax over keys m)
    y     = g @ attn^T                 [64, N]
    out   = w_last @ y + xf            [128, N]

Design (per core), v6:
 - scoresT[m, q] orientation (phi tiles stationary) so exp(scoresT)
   feeds the y matmul directly as the moving operand.
 - The PE moving-operand path is ~256 B/cycle; fp16 theta/phi halve
   the bytes per column, so the two concurrent 64-row score matmuls
   (m-tile pair in disjoint PE row groups) stream 2 cols/cycle --
   2x over f32r at any clock. fp16's 10-bit mantissa keeps the
   logit error ~10x below bf16's.
 - exp split 20/12 between ACT (real Exp) and DVE (Schraudolph
   bit-trick: one tensor_scalar affine with int16 output bitcast as
   bf16). The y/w_last matmuls are uniform bf16 (range needs bf16:
   unnormalized y and row sums reach e^44).
 - Row sums via a ones column in gt; normalization: DVE reciprocal
   straight off PSUM + gpsimd partition_broadcast; the w_last matmul
   of chunk c is deferred into chunk c+1's PE stream.
 - Input DMA / projections / first q-chunk fused so the PE starts
   while the input streams in; weights DMA'd before the bulk input.
"""

import sys

import numpy as np

for _p in ("/opt/trn_rl_repo",):
    if _p not in sys.path:
        sys.path.insert(0, _p)

import concourse.bass as bass
from concourse import bacc
import concourse.mybir as mybir
import concourse.tile as tile
from concourse.alu_op_type import AluOpType
from concourse.bass_utils import run_bass_kernel_spmd

F32 = mybir.dt.float32
F16 = mybir.dt.float16
BF16 = mybir.dt.bfloat16
I16 = mybir.dt.int16

P = 128     # channels C / partition dim
CB = 64     # bottleneck channels
NQ = 4096   # spatial positions (64*64)
NMT = 32    # m (key) tiles of 128
NPAIR = 16  # m-tile pairs per q chunk
NQC = 8     # q chunks of 512

# Schraudolph exp in bf16: exp(s) ~= bitcast<bf16>(int16(A*s + B))
EXP_A = float(2**7 / np.log(2.0))
EXP_B = float(127.0 * 2**7 - 6.0)
# m-tiles with mi % 8 < ACT_SHARE go to the ACT engine, rest to DVE
ACT_SHARE = 5

_NC_CACHE = {}


def _build():
    nc = bacc.Bacc()
    x_in = nc.declare_dram_parameter("xb", [P, NQ], F32, isOutput=False)
    wqk_in = nc.declare_dram_parameter("wqk", [P, P], F32, isOutput=False)
    wg_in = nc.declare_dram_parameter("wgT", [P, CB], F32, isOutput=False)
    wl_in = nc.declare_dram_parameter("wl", [CB, P], F32, isOutput=False)
    out_d = nc.declare_dram_parameter("out", [P, NQ], F32, isOutput=True)

    with tile.TileContext(nc) as tc:
        with (
            tc.tile_pool(name="const", bufs=1) as const,
            tc.tile_pool(name="big", bufs=1) as big,
            tc.tile_pool(name="work", bufs=2) as work,
            tc.tile_pool(name="probs", bufs=6) as probs,
            tc.tile_pool(name="spool", bufs=6, space="PSUM") as spool,
            tc.tile_pool(name="ypool", bufs=2, space="PSUM") as ypool,
        ):
            # ---- small weights first (needed before any compute) ----
            wqk_f = const.tile([P, P], F32)
            wg_f = const.tile([P, CB], F32)
            wl_f = const.tile([CB, P], F32)
            nc.sync.dma_start(out=wqk_f, in_=wqk_in[:, :])
            nc.sync.dma_start(out=wg_f, in_=wg_in[:, :])
            nc.sync.dma_start(out=wl_f, in_=wl_in[:, :])
            wqk = const.tile([P, P], F16)
            wg = const.tile([P, CB], F16)
            wl = const.tile([CB, P], BF16)
            nc.vector.tensor_copy(wqk, wqk_f)
            nc.vector.tensor_copy(wg, wg_f)
            nc.vector.tensor_copy(wl, wl_f)

            xb = big.tile([P, NQ], F32)
            xb16 = big.tile([P, NQ], F16)
            # theta/phi duplicated into both partition halves so score
            # matmuls for two m-tiles run concurrently in PE row groups
            theta = big.tile([P, NQ], F16)
            phi = big.tile([P, NQ], F16)
            # gT in 65-col slots; col 64 = ones for the row-sum trick
            gt = big.tile([P, NMT * (CB + 1)], BF16)
            nc.vector.memset(gt, 1.0)
            gt3 = gt.rearrange("p (m c) -> p m c", c=CB + 1)

            # ---------------- pipelined helpers ----------------
            qof = [qc * 512 for qc in range(NQC)]

            def score_pair(qc, pi):
                """Two concurrent 64-row score matmuls for m-tiles
                2*pi (rows 0:64) and 2*pi+1 (rows 64:128)."""
                q = qof[qc]
                sa = spool.tile([P, 512], F32, tag="s")
                nc.tensor.matmul(
                    sa, phi[0:CB, (2 * pi) * 128:(2 * pi + 1) * 128],
                    theta[0:CB, q:q + 512], start=True, stop=True,
                )
                sb = spool.tile([P, 512], F32, tag="s")
                nc.tensor.matmul(
                    sb, phi[CB:P, (2 * pi + 1) * 128:(2 * pi + 2) * 128],
                    theta[CB:P, q:q + 512], start=True, stop=True,
                )
                return sa, sb

            def exp_tile(mi, sp):
                pb = probs.tile([P, 512], BF16, tag="pb")
                if mi % 8 < ACT_SHARE:
                    nc.scalar.activation(
                        pb, sp, mybir.ActivationFunctionType.Exp
                    )
                else:
                    nc.vector.tensor_scalar(
                        pb.bitcast(I16), sp, EXP_A, EXP_B,
                        AluOpType.mult, AluOpType.add,
                    )
                return pb

            def y_mm(yps, mi, pb):
                nc.tensor.matmul(
                    yps,
                    gt[:, mi * (CB + 1):(mi + 1) * (CB + 1)],
                    pb,
                    start=(mi == 0), stop=(mi == NMT - 1),
                )

            # per-chunk epilogue, split so the w_last matmul can be
            # deferred into the next chunk's PE stream
            def epi_head(qc, yps):
                rinv = work.tile([1, 512], F32, tag="rinv")
                nc.vector.reciprocal(rinv, yps[CB:CB + 1, :])
                yu = work.tile([CB, 512], BF16, tag="yu")
                nc.vector.tensor_copy(yu, yps[0:CB, :])  # frees yps slot
                rb = work.tile([P, 512], F32, tag="rb")
                nc.gpsimd.partition_broadcast(rb, rinv)
                return yu, rb

            def epi_mm(state):
                qc, yu, rb = state
                op = spool.tile([P, 512], F32, tag="s")
                nc.tensor.matmul(op, wl, yu, start=True, stop=True)
                return (qc, op, rb)

            def epi_tail(state):
                qc, op, rb = state
                ob = work.tile([P, 512], F32, tag="ob")
                nc.vector.tensor_mul(ob, op, rb)
                ob2 = work.tile([P, 512], F32, tag="ob2")
                nc.vector.tensor_add(ob2, ob, xb[:, qof[qc]:qof[qc] + 512])
                nc.sync.dma_start(out=out_d[:, qof[qc]:qof[qc] + 512], in_=ob2)

            def pair_tiles(qc, yps, pi, look=2):
                """Scores+exp for pair pi, y matmuls for pair pi-look."""
                sa, sb = score_pair(qc, pi)
                pbq[2 * pi] = exp_tile(2 * pi, sa)
                pbq[2 * pi + 1] = exp_tile(2 * pi + 1, sb)
                pj = pi - look
                if pj >= 0:
                    for mi in (2 * pj, 2 * pj + 1):
                        y_mm(yps, mi, pbq.pop(mi))

            def drain_y(yps, look=2):
                for pj in range(NPAIR - look, NPAIR):
                    for mi in (2 * pj, 2 * pj + 1):
                        y_mm(yps, mi, pbq.pop(mi))

            # ---------------- init fused with q-chunk 0 ----------------
            # Per 512-col xb chunk j: DMA, theta/phi projection, 4 gT
            # projections; from j>=1 also run q-chunk-0 score/exp/y for
            # the m-tile pairs whose phi/gt landed in iteration j-1.
            yps0 = ypool.tile([CB + 1, 512], F32, tag="y")
            pbq = {}  # mi -> pb tile awaiting its y matmul

            for j in range(8):
                cs = slice(j * 512, (j + 1) * 512)
                nc.sync.dma_start(out=xb[:, cs], in_=x_in[:, cs])
                nc.scalar.copy(xb16[:, cs], xb[:, cs])
                ps = spool.tile([P, 512], F32, tag="s")
                nc.tensor.matmul(ps, wqk, xb16[:, cs], start=True, stop=True)
                # theta lower half is partition-aligned -> ACT engine;
                # the shifted copies (DVE only) fill the other halves
                nc.scalar.copy(theta[0:CB, cs], ps[0:CB, :])
                nc.vector.tensor_copy(phi[0:CB, cs], ps[CB:P, :])
                nc.vector.tensor_copy(phi[CB:P, cs], ps[CB:P, :])
                if j == 0:
                    nc.vector.tensor_copy(theta[CB:P, cs], ps[0:CB, :])
                gp = spool.tile([P, 512], F32, tag="s")
                gp3 = gp.rearrange("p (m c) -> p m c", c=CB)
                for k in range(4):
                    mi = 4 * j + k
                    nc.tensor.matmul(
                        gp3[:, k, :], xb16[:, mi * 128:(mi + 1) * 128], wg,
                        start=True, stop=True,
                    )
                nc.scalar.copy(gt3[:, 4 * j:4 * j + 4, 0:CB], gp3[:, 0:4, :])
                if j >= 1:
                    for pi in (2 * (j - 1), 2 * (j - 1) + 1):
                        pair_tiles(0, yps0, pi)
            for pi in (14, 15):
                pair_tiles(0, yps0, pi)
            drain_y(yps0)
            # upper-theta for chunk 1 (needed at its start)
            c1 = slice(512, 1024)
            nc.vector.tensor_copy(theta[CB:P, c1], theta[0:CB, c1])
            pending = (0,) + epi_head(0, yps0)

            # ---------------- q-chunks 1..7 ----------------
            for qc in range(1, NQC):
                yps = ypool.tile([CB + 1, 512], F32, tag="y")
                for pi in range(NPAIR):
                    pair_tiles(qc, yps, pi)
                    if pi == 2 and pending is not None:
                        pending = epi_mm(pending)
                    if pi == 3 and pending is not None:
                        epi_tail(pending)
                        pending = None
                    if pi == 8 and qc < NQC - 1:
                        # upper-theta for the next chunk, off-peak on DVE
                        cn = slice(qof[qc + 1], qof[qc + 1] + 512)
                        nc.vector.tensor_copy(theta[CB:P, cn], theta[0:CB, cn])
                drain_y(yps)
                pending = (qc,) + epi_head(qc, yps)

            epi_tail(epi_mm(pending))

    nc.finalize()
    return nc


def kernel(x, w_theta, w_phi, w_g, w_last):
    B, C, H, W = x.shape
    N = H * W
    xf = np.ascontiguousarray(x.reshape(B, C, N), dtype=np.float32)
    wqk = np.ascontiguousarray(
        np.concatenate([w_theta.T, w_phi.T], axis=1), dtype=np.float32
    )
    wgT = np.ascontiguousarray(w_g.T, dtype=np.float32)
    wl = np.ascontiguousarray(w_last.T, dtype=np.float32)

    if "nc" not in _NC_CACHE:
        _NC_CACHE["nc"] = _build()
    nc = _NC_CACHE["nc"]

    in_maps = [
        {"xb": xf[b], "wqk": wqk, "wgT": wgT, "wl": wl} for b in range(B)
    ]
    r = run_bass_kernel_spmd(nc, in_maps, list(range(B)))
    out = np.stack([r.results[b]["out"] for b in range(B)], axis=0)
    return out.reshape(B, C, H, W).astype(np.float32)
